# revision 52
# baseline (speedup 1.0000x reference)
"""BERT self-attention on 8 Trainium2 NeuronCores (Bass/Tile).

Problem: B=8, S=1024, H=1024, NH=16, HD=64, fp32.
Sharding: pure data-parallel — one batch element per core, weights
replicated. No collectives.

Math notes:
- The attention-mask bias broadcasts over keys ((1-mask)[...,None] is a
  per-(batch,query) constant added to every logit of a softmax row), so
  it cancels exactly in softmax for any finite mask. It is not used.
- Softmax is computed without max-subtraction: logits are ~N(0,1)
  (|max| < ~6), exp is comfortably within fp32 range.

Design (252.5us -> 180.4us vs the PE-transpose/fp32 v1):
- x and Wq/Wk/Wv are pre-transposed AND converted to bf16 on the host:
  xT[h,s] / wT[h,o] land in DRAM so DMA loads them straight into the
  [contraction-on-partitions] layout. No PE transposes at all (v1 spent
  ~35us of PE on transposes + ~42us of DVE on PSUM->SBUF copies).
- PV runs in natural layout: lhsT = E-chunk (bf16 stationary,
  [k=128, q=128]), rhs = Vpad[k, 65] (bf16 moving, N=65 -> 65 cycles
  at 1 cyc/row; the cost model charges out-free-size x cyc/row of the
  MOVING dtype, K/M are free). ctx comes out [q, d] natural — no ctx
  transposes — and the softmax denominator column rides along.
- Q/K projections and scores keep fp32r accumulate layouts (moving
  N=512 >= 256 -> 1 cyc/row). Measured rel err ~5e-3 vs the 2e-2 gate.
- PE work is at the streamed-column floor (~165us: proj 82 + scores
  54.6 + PV 27.5 + V included in proj); the schedule's job is keeping
  PE gapless against ACT's softmax-exp pacing (1038ns/slot vs 426ns of
  scores PE per slot).
- Fully software-pipelined slot schedule: every exp "slot" carries
  filler PE work — PV chunk-groups, split proj halves, and the NEXT
  o-tile's Q0/K0 projections — so there is no per-ot serial phase.
  Weight slices prefetch one ot ahead on SP before store waits queue.

Per-ot slot layout (steady state, ot >= 3; pvP = PV of (ot-1, qb1),
pvC = PV of (ot, qb0), a/b = 4+4 ht split, chunk-pair granularity):
  qb0: K1a | K1b | pvP.j0.c01 | Q1a | Q1b | pvP.j0.c23 | pvP.j1.c01
       | pvP.j1.c23
  qb1: Q0n.a | Q0n.b | pvC.j0.c01 | K0n.a | K0n.b | pvC.j0.c23
       | pvC.j1.c01 | pvC.j1.c23
ot0 fills with the 16 V units (minus 8 blk1 units that spill into
ot1/ot2's spare slots); the Vpad ones column is data-independent and
filled before the loop. The last ot pre-runs the final pair's PV into
4 one-bank PSUM tiles ([P, 2, 65], one accumulation group each) behind
the closing exps, leaving only the kt7 matmuls + normalize + one fused
store after the last exp.
"""
import numpy as np
from contextlib import ExitStack

import concourse.bass as bass
import concourse.tile as tile
from concourse import bacc, mybir
from concourse.bass_utils import run_bass_kernel_spmd

B, S, H, NH = 8, 1024, 1024, 16
HD = H // NH          # 64
P = 128
NT = S // P           # 8 s-tiles
HT = H // P           # 8 h-tiles (contraction)
OT = H // P           # 8 o-tiles / head pairs
QBS = 512             # q-block size
NQB = S // QBS        # 2 q-blocks
NC_ = QBS // P        # 4 q-chunks per block
N_CORES = 8
F32 = mybir.dt.float32
F32R = mybir.dt.float32r
BF16 = mybir.dt.bfloat16
F8 = mybir.dt.float8e4
AF = mybir.ActivationFunctionType
ALU = mybir.AluOpType
VW = HD + 1           # V unit cols: 64 d + ones (denominator) col

_CACHE = {}

TUNE = {
    "pv_bufs": 4,      # 1-bank psum slots for proj-halves / PV / V units
    "s_bufs": 2,       # 2-bank psum slots for scores (exp-paced)
}


def _emit(tc):
    nc = tc.nc
    xh = nc.dram_tensor("xh", [H, S], F8, kind="ExternalInput").ap()
    xl = nc.dram_tensor("xl", [H, S], F8, kind="ExternalInput").ap()
    wqh = nc.dram_tensor("wqh", [H, H], F8, kind="ExternalInput").ap()
    wql = nc.dram_tensor("wql", [H, H], F8, kind="ExternalInput").ap()
    wkh = nc.dram_tensor("wkh", [H, H], F8, kind="ExternalInput").ap()
    wkl = nc.dram_tensor("wkl", [H, H], F8, kind="ExternalInput").ap()
    wvh = nc.dram_tensor("wvh", [H, H], F8, kind="ExternalInput").ap()
    wvl = nc.dram_tensor("wvl", [H, H], F8, kind="ExternalInput").ap()
    bq = nc.dram_tensor("bq", [H], F32, kind="ExternalInput").ap()
    bk = nc.dram_tensor("bk", [H], F32, kind="ExternalInput").ap()
    bv = nc.dram_tensor("bv", [H], F32, kind="ExternalInput").ap()
    out = nc.dram_tensor("out", [S, H], F32, kind="ExternalOutput").ap()

    with ExitStack() as top:
        consts = top.enter_context(tc.tile_pool(name="consts", bufs=1))
        big = top.enter_context(tc.tile_pool(name="big", bufs=1))
        wt = top.enter_context(tc.tile_pool(name="wt", bufs=8))
        wtv = top.enter_context(tc.tile_pool(name="wtv", bufs=4))
        qk = top.enter_context(tc.tile_pool(name="qk", bufs=4))
        cp = top.enter_context(tc.tile_pool(name="cp", bufs=4))
        ep = top.enter_context(tc.tile_pool(name="ep", bufs=2))

        # XTH/XTL[p, ht, s] = fp8 hi/lo split of x[s, ht*P+p]
        XTH = big.tile([P, HT, S], F8, tag="XTH")
        XTL = big.tile([P, HT, S], F8, tag="XTL")
        Vpad = big.tile([P, NT, NH, VW], BF16, tag="Vpad")

        xh_t = xh.rearrange("(t p) s -> p t s", p=P)
        xl_t = xl.rearrange("(t p) s -> p t s", p=P)
        wq_t = (wqh.rearrange("(t p) o -> p t o", p=P),
                wql.rearrange("(t p) o -> p t o", p=P))
        wk_t = (wkh.rearrange("(t p) o -> p t o", p=P),
                wkl.rearrange("(t p) o -> p t o", p=P))
        wv_t = (wvh.rearrange("(t p) o -> p t o", p=P),
                wvl.rearrange("(t p) o -> p t o", p=P))

        with ExitStack() as phb:
            ps_s = phb.enter_context(
                tc.tile_pool(name="ps_s", bufs=TUNE["s_bufs"], space="PSUM"))
            ps_pv = phb.enter_context(
                tc.tile_pool(name="ps_pv", bufs=TUNE["pv_bufs"], space="PSUM"))

            def load_w(w_t, ot):
                wnh = wt.tile([P, HT, P], F8, tag="wt", name="wnh")
                nc.sync.dma_start(wnh[:], w_t[0][:, :, ot * P:(ot + 1) * P])
                wnl = wt.tile([P, HT, P], F8, tag="wt", name="wnl")
                nc.sync.dma_start(wnl[:], w_t[1][:, :, ot * P:(ot + 1) * P])
                return (wnh, wnl)

            def load_wv_block(blk):
                # hi/lo pair; ht-half DMAs so a V unit's early matmuls can
                # start while the rest is still in flight
                cs = slice(blk * 4 * P, (blk + 1) * 4 * P)
                sbh = wtv.tile([P, HT, 4 * P], F8, tag="wtv", name="sbh")
                nc.sync.dma_start(sbh[:, 0:4, :], wv_t[0][:, 0:4, cs])
                nc.sync.dma_start(sbh[:, 4:, :], wv_t[0][:, 4:, cs])
                sbl = wtv.tile([P, HT, 4 * P], F8, tag="wtv", name="sbl")
                nc.sync.dma_start(sbl[:, 0:4, :], wv_t[1][:, 0:4, cs])
                nc.sync.dma_start(sbl[:, 4:, :], wv_t[1][:, 4:, cs])
                return (sbh, sbl)

            # ---- startup DMA order: first proj's inputs first -------------
            # wq0-hi ht-pair0 (tiny) -> x-hi pair0 -> the rest; XTL comes
            # after XTH because the mm term order puts (whi, XTL) last
            wq0h = wt.tile([P, HT, P], F8, tag="wt", name="wq0h")
            nc.sync.dma_start(wq0h[:, 0:2, :], wq_t[0][:, 0:2, 0:P])
            nc.sync.dma_start(XTH[:, 0:2, 0:QBS], xh_t[:, 0:2, 0:QBS])
            nc.sync.dma_start(wq0h[:, 2:, :], wq_t[0][:, 2:, 0:P])
            wq0l = wt.tile([P, HT, P], F8, tag="wt", name="wq0l")
            nc.sync.dma_start(wq0l[:], wq_t[1][:, :, 0:P])
            wq0 = (wq0h, wq0l)
            nc.sync.dma_start(XTH[:, 2:4, 0:QBS], xh_t[:, 2:4, 0:QBS])
            nc.sync.dma_start(XTH[:, 4:, 0:QBS], xh_t[:, 4:, 0:QBS])
            nc.sync.dma_start(XTL[:, :, 0:QBS], xl_t[:, :, 0:QBS])
            wk0 = load_w(wk_t, 0)
            bq_sb = consts.tile([P, OT], F32, tag="bq")
            nc.sync.dma_start(bq_sb[:], bq.rearrange("(t p) -> p t", p=P))
            bk_sb = consts.tile([P, OT], F32, tag="bk")
            nc.sync.dma_start(bk_sb[:], bk.rearrange("(t p) -> p t", p=P))
            bv_row = consts.tile([1, H], F32, tag="bv_row")
            nc.sync.dma_start(bv_row[:], bv.unsqueeze(0))
            wv_blk = [load_wv_block(0), None]
            # x s-half1: first needed by K-half1 (ot0 qb0-kt2) / V(st>=4)
            nc.sync.dma_start(XTH[:, :, QBS:S], xh_t[:, :, QBS:S])
            nc.sync.dma_start(XTL[:, :, QBS:S], xl_t[:, :, QBS:S])

            bv_bc = consts.tile([P, H], F32, tag="bv_bc")
            nc.gpsimd.partition_broadcast(bv_bc[:], bv_row[:])
            ones_f32 = consts.tile([P, P], F32, tag="ones")
            nc.vector.memset(ones_f32[:], 32.0)
            # Vpad ones + pad columns are data-independent: fill them now so
            # PV(ot0, qb0) can weave into ot0's qb1 stretch
            nc.vector.tensor_copy(
                Vpad[:, :, :, HD],
                ones_f32[:].rearrange("p (a b) -> p a b", a=NT))

            DR = mybir.MatmulPerfMode.DoubleRow

            def proj_seg(wpair, sb, acc, seg):
                # x@W via fp8 DoubleRow (ht-pairs folded, 256 rows/pass):
                # terms xh@wh + xl@wh + xh@wl; the xl@wl term is ~0.1% and
                # dropped. 12 matmuls x 256 cyc vs bf16's 8 x 512.
                whi, wlo = wpair
                mms = [(w, x, tp)
                       for (w, x) in ((whi, XTH), (wlo, XTH), (whi, XTL))
                       for tp in range(HT // 2)]
                lo, hi = (0, 6) if seg == 0 else (6, 12)
                for idx in range(lo, hi):
                    w, x, tp = mms[idx]
                    nc.tensor.matmul(
                        acc[:], w[:, 2 * tp:2 * tp + 2, :],
                        x[:, 2 * tp:2 * tp + 2, sb * QBS:(sb + 1) * QBS],
                        start=(idx == 0), stop=(idx == 11), perf_mode=DR)

            def proj_finish(sb, acc, dst, bias_sb, ot):
                nc.vector.tensor_scalar_add(
                    dst[:, sb * QBS:(sb + 1) * QBS], acc[:], bias_sb[:, ot:ot + 1])

            def proj_half(wpair, sb, dst, bias_sb, ot):
                acc = ps_pv.tile([P, QBS], F32, tag="pv")
                proj_seg(wpair, sb, acc, 0)
                proj_seg(wpair, sb, acc, 1)
                proj_finish(sb, acc, dst, bias_sb, ot)

            def emit_v_unit(hp, st):
                # one s-tile of V for ONE head pair hp (heads 2hp, 2hp+1):
                # 320ns of PE — fine-grained enough to spread so each ot
                # carries only its own head pair's V work just in time
                blk = hp // 4
                wvh_sb, wvl_sb = wv_blk[blk]
                cs = slice((hp % 4) * P, (hp % 4 + 1) * P)
                vm = ps_pv.tile([P, P], F32, tag="pv", name="vm")
                mms = [(x, w, tp)
                       for (x, w) in ((XTH, wvh_sb), (XTL, wvh_sb),
                                      (XTH, wvl_sb))
                       for tp in range(HT // 2)]
                for idx, (x, w, tp) in enumerate(mms):
                    nc.tensor.matmul(
                        vm[:], x[:, 2 * tp:2 * tp + 2, st * P:(st + 1) * P],
                        w[:, 2 * tp:2 * tp + 2, cs],
                        start=(idx == 0), stop=(idx == len(mms) - 1),
                        perf_mode=DR)
                nc.vector.tensor_tensor(
                    Vpad[:, st, 2 * hp:2 * hp + 2, 0:HD],
                    vm[:].rearrange("p (h d) -> p h d", d=HD),
                    bv_bc[:, hp * P:(hp + 1) * P].rearrange(
                        "p (h d) -> p h d", d=HD),
                    ALU.add)

            out_tiled = out.rearrange("(t p) o -> p t o", p=P)

            def emit_pv_chunks(ot, qb, E, j, c_lo, c_hi, ctb):
                # ctx[q-chunk, 0:64] + denom col: lhsT = E-chunk (bf16,
                # stationary), rhs = Vpad[k, 66] (moving, N=66). ctx chunks
                # stage into ctb; the caller stores after the last chunk.
                h = 2 * ot + j
                for c in range(c_lo, c_hi):
                    pv = ps_pv.tile([P, VW], F32, tag="pv")
                    for kt in range(NT):
                        nc.tensor.matmul(
                            pv[:], E[:, kt, j, c * P:(c + 1) * P],
                            Vpad[:, kt, h, :],
                            start=(kt == 0), stop=(kt == NT - 1))
                    rc = cp.tile([P, 1], F32, tag="rc")
                    nc.vector.reciprocal(rc[:], pv[:, HD:HD + 1])
                    nc.vector.tensor_scalar_mul(ctb[:, c, :], pv[:, 0:HD], rc[:])
                if c_hi == NC_:
                    nc.sync.dma_start(
                        out_tiled[:, qb * NC_:(qb + 1) * NC_,
                                  h * HD:(h + 1) * HD],
                        ctb[:])

            def emit_pv_one(ot, qb, E, j):
                ctb = cp.tile([P, NC_, HD], F32, tag="ctb")
                emit_pv_chunks(ot, qb, E, j, 0, NC_, ctb)

            def pv_half_emitters(ot, qb, E, j):
                # two half-unit thunks (2 chunks each) sharing one ctb,
                # allocated lazily at the first thunk's emission point
                box = [None]

                def a():
                    box[0] = cp.tile([P, NC_, HD], F32, tag="ctb",
                                     name="ctb")
                    emit_pv_chunks(ot, qb, E, j, 0, 2, box[0])

                def b():
                    emit_pv_chunks(ot, qb, E, j, 2, NC_, box[0])
                return a, b

            def scores_slot(E, qt, kt_, qb, kt, filler=None):
                ss = ps_s.tile([P, 2, QBS], F32, tag="s")
                for j in range(2):
                    pr = slice(j * HD, (j + 1) * HD)
                    nc.tensor.matmul(
                        ss[:, j, :],
                        kt_[pr, kt * P:(kt + 1) * P],
                        qt[pr, qb * QBS:(qb + 1) * QBS],
                        start=True, stop=True)
                nc.scalar.activation(E[:, kt, :, :], ss[:], AF.Exp, scale=0.125 / 1024.0)
                if filler is not None:
                    filler()

            # ---- software-pipelined ot loop -------------------------------
            # state carried across iterations:
            qt = qk.tile([P, S], F32R, tag="qt")
            kt_ = qk.tile([P, S], F32R, tag="kt")
            proj_half(wq0, 0, qt, bq_sb, 0)      # ot0 Q-half0 (not woven)
            proj_half(wk0, 0, kt_, bk_sb, 0)     # ot0 K-half0
            pv_prev = None                        # (ot-1, 1, E1)

            # V(hp) units: hp0 fills ot0-qb0; hp_j (j>=1) runs 4 units
            # in ot_{j-1}-qb1 and 4 in ot_j-qb0 — always complete before
            # pvC(ot_j) reads them at ot_j-qb1-kt2
            tail_box = [None]

            for ot in range(OT):
                # prefetch next ot's weight slices before any stores enqueue
                # on SP this iteration
                if ot == 1:
                    wv_blk[1] = load_wv_block(1)
                if ot + 1 < OT:
                    wq_n = load_w(wq_t, ot + 1)
                    wk_n = load_w(wk_t, ot + 1)
                wTq = wq0 if ot == 0 else wq_cur
                wTk = wk0 if ot == 0 else wk_cur

                E0 = ep.tile([P, NT, 2, QBS], BF16, tag="E")
                E1 = ep.tile([P, NT, 2, QBS], BF16, tag="E")

                # --- qb0 stretch -----------------------------------------
                k1_acc = [None]
                q1_acc = [None]

                def f_k1a(wTk=wTk, k1_acc=k1_acc):
                    k1_acc[0] = ps_pv.tile([P, QBS], F32, tag="pv", name="k1_acc")
                    proj_seg(wTk, 1, k1_acc[0], 0)

                def f_k1b(wTk=wTk, kt_=kt_, ot=ot, k1_acc=k1_acc):
                    proj_seg(wTk, 1, k1_acc[0], 1)
                    proj_finish(1, k1_acc[0], kt_, bk_sb, ot)

                def f_q1a(wTq=wTq, q1_acc=q1_acc):
                    q1_acc[0] = ps_pv.tile([P, QBS], F32, tag="pv", name="q1_acc")
                    proj_seg(wTq, 1, q1_acc[0], 0)

                def f_q1b(wTq=wTq, qt=qt, ot=ot, q1_acc=q1_acc):
                    proj_seg(wTq, 1, q1_acc[0], 1)
                    proj_finish(1, q1_acc[0], qt, bq_sb, ot)

                if ot == 0:
                    # all 8 V(hp0) units; K1 at kt2 (completes before the
                    # kt4-7 scores that read K-half1), Q1 at kt5 (before qb1)
                    def qb0_filler(kt):
                        if kt == 2:
                            f_k1a(); f_k1b()
                        elif kt == 5:
                            f_q1a(); f_q1b()
                        emit_v_unit(0, kt)
                else:
                    # pvP reads the PREVIOUS pair's E1 whose last exps drain
                    # ~2 slots into this stretch: keep kt0/kt1 exp-independent.
                    # kts 0-3 also carry this ot's V(hp_ot, st4-7) units.
                    pP0a, pP0b = pv_half_emitters(*pv_prev, 0)
                    pP1a, pP1b = pv_half_emitters(*pv_prev, 1)

                    def qb0_filler(kt, ot=ot, fns=(f_k1a, f_k1b, pP0a, f_q1a,
                                                   f_q1b, pP0b, pP1a, pP1b)):
                        if kt < 4:
                            emit_v_unit(ot, 4 + kt)
                        fns[kt]()

                for kt in range(NT):
                    scores_slot(E0, qt, kt_, 0, kt,
                                (lambda kt=kt: qb0_filler(kt)))

                # --- qb1 stretch -----------------------------------------
                if ot + 1 < OT:
                    qt_n = qk.tile([P, S], F32R, tag="qt")
                    kt_n = qk.tile([P, S], F32R, tag="kt")
                    q0n_acc = [None]
                    k0n_acc = [None]

                    def f_q0na(wq_n=wq_n, q0n_acc=q0n_acc):
                        q0n_acc[0] = ps_pv.tile([P, QBS], F32, tag="pv", name="q0n_acc")
                        proj_seg(wq_n, 0, q0n_acc[0], 0)

                    def f_q0nb(wq_n=wq_n, qt_n=qt_n, ot=ot, q0n_acc=q0n_acc):
                        proj_seg(wq_n, 0, q0n_acc[0], 1)
                        proj_finish(0, q0n_acc[0], qt_n, bq_sb, ot + 1)

                    def f_k0na(wk_n=wk_n, k0n_acc=k0n_acc):
                        k0n_acc[0] = ps_pv.tile([P, QBS], F32, tag="pv", name="k0n_acc")
                        proj_seg(wk_n, 0, k0n_acc[0], 0)

                    def f_k0nb(wk_n=wk_n, kt_n=kt_n, ot=ot, k0n_acc=k0n_acc):
                        proj_seg(wk_n, 0, k0n_acc[0], 1)
                        proj_finish(0, k0n_acc[0], kt_n, bk_sb, ot + 1)
                else:
                    qt_n = kt_n = None
                    f_q0na = f_q0nb = f_k0na = f_k0nb = (lambda: None)

                if ot < OT - 1:
                    # even spread: pvC at half-unit granularity (E0 is fully
                    # drained once qb1 starts)
                    pC0a, pC0b = pv_half_emitters(ot, 0, E0, 0)
                    pC1a, pC1b = pv_half_emitters(ot, 0, E0, 1)

                    def qb1_filler(kt, ot=ot, fns=(f_q0na, f_q0nb, pC0a,
                                                   f_k0na, f_k0nb, pC0b,
                                                   pC1a, pC1b)):
                        if kt < 4:
                            emit_v_unit(ot + 1, kt)
                        fns[kt]()
                else:
                    # last ot: pvC moves to the early slots. The final
                    # pair's PV (both heads) accumulates into 4 PSUM tiles
                    # of [P, 2j, VW] (1 bank each) whose kt-matmuls pre-run
                    # in kt4-7 right behind the closing exps — only the kt7
                    # matmuls + normalize + 2 stores remain after the last
                    # exp.
                    pC0a, pC0b = pv_half_emitters(ot, 0, E0, 0)
                    pC1a, pC1b = pv_half_emitters(ot, 0, E0, 1)
                    tl_pv = [None] * NC_
                    tl_ctb = [None]

                    def tail_mms(E1=E1, ot=ot, kts=()):
                        # both heads share one accumulation group per chunk
                        # (one PSUM zero-region): start zeroes the region at
                        # (j0, kt0); stop closes it at (j1, kt7)
                        for c in range(NC_):
                            for kt in kts:
                                for j in range(2):
                                    nc.tensor.matmul(
                                        tl_pv[c][:, j, :],
                                        E1[:, kt, j, c * P:(c + 1) * P],
                                        Vpad[:, kt, 2 * ot + j, :],
                                        start=(kt == 0 and j == 0),
                                        stop=(kt == NT - 1 and j == 1))

                    def tail_start():
                        for c in range(NC_):
                            tl_pv[c] = ps_pv.tile([P, 2, VW], F32, tag="pv",
                                                  name="tl_pv")
                        # both heads' ctx per chunk, contiguous out columns:
                        # one 512B-row store per chunk as soon as it's ready
                        tl_ctb[0] = cp.tile([P, NC_, 2 * HD], F32, tag="ctb",
                                            name="tl_ctb")
                        tail_mms(kts=range(0, 4))

                    def tail_finish(ot=ot, E1=E1):
                        # close all groups, then all reciprocals, then the
                        # normalizes split DVE(j0)/ACT(j1), then one fused
                        # store of both heads x 4 chunks
                        tail_mms(kts=range(NT - 1, NT))
                        rcs = cp.tile([P, NC_, 2], F32, tag="rc",
                                      name="rcs")
                        for c in range(NC_):
                            for j in range(2):
                                nc.vector.reciprocal(
                                    rcs[:, c, j:j + 1],
                                    tl_pv[c][:, j, HD:HD + 1])
                        for c in range(NC_):
                            nc.vector.tensor_scalar_mul(
                                tl_ctb[0][:, c, 0:HD],
                                tl_pv[c][:, 0, 0:HD], rcs[:, c, 0:1])
                            nc.scalar.activation(
                                tl_ctb[0][:, c, HD:2 * HD],
                                tl_pv[c][:, 1, 0:HD],
                                AF.Copy, scale=rcs[:, c, 1:2])
                        nc.sync.dma_start(
                            out_tiled[:, NC_:2 * NC_,
                                      2 * ot * HD:(2 * ot + 2) * HD],
                            tl_ctb[0][:])

                    tail_box[0] = tail_finish

                    def qb1_filler(kt):
                        if kt == 0:
                            pC0a()
                        elif kt == 1:
                            pC0b()
                        elif kt == 2:
                            pC1a()
                        elif kt == 3:
                            pC1b()
                        elif kt == 4:
                            tail_start()
                        elif kt == 5:
                            tail_mms(kts=range(4, 6))
                        elif kt == 6:
                            tail_mms(kts=range(6, 7))

                for kt in range(NT):
                    scores_slot(E1, qt, kt_, 1, kt,
                                (lambda kt=kt: qb1_filler(kt)))

                pv_prev = (ot, 1, E1)
                if ot + 1 < OT:
                    wq_cur, wk_cur = wq_n, wk_n
                    qt, kt_ = qt_n, kt_n

            # tail: last kt-matmuls + normalize + stores of the final pair
            tail_box[0]()


def build():
    if "nc" in _CACHE:
        return _CACHE["nc"]
    nc = bacc.Bacc("TRN2", target_bir_lowering=False, debug=False,
                   num_devices=N_CORES)
    with tile.TileContext(nc) as tc:
        _emit(tc)
    nc.compile()
    _CACHE["nc"] = nc
    return nc


WSCALE = 32.0  # weights pre-scaled into e4m3's normal range; the exp
                # scale (/1024) and the Vpad ones column (=32) compensate


def _hi_lo(a):
    import ml_dtypes
    f8 = ml_dtypes.float8_e4m3
    hi = a.astype(f8)
    lo = (a - hi.astype(np.float32)).astype(f8)
    return np.ascontiguousarray(hi), np.ascontiguousarray(lo)


def make_in_maps(hidden_state, Wq, bq, Wk, bk, Wv, bv):
    hs = np.asarray(hidden_state, dtype=np.float32)
    wqh, wql = _hi_lo(np.asarray(Wq, np.float32).T * WSCALE)
    wkh, wkl = _hi_lo(np.asarray(Wk, np.float32).T * WSCALE)
    wvh, wvl = _hi_lo(np.asarray(Wv, np.float32).T * WSCALE)
    common = {
        "wqh": wqh, "wql": wql, "wkh": wkh, "wkl": wkl,
        "wvh": wvh, "wvl": wvl,
        "bq": np.ascontiguousarray(np.asarray(bq, np.float32) * WSCALE),
        "bk": np.ascontiguousarray(np.asarray(bk, np.float32) * WSCALE),
        "bv": np.ascontiguousarray(np.asarray(bv, np.float32) * WSCALE),
    }
    maps = []
    for i in range(N_CORES):
        xh_, xl_ = _hi_lo(np.ascontiguousarray(hs[i].T))
        maps.append({"xh": xh_, "xl": xl_, **common})
    return maps


def kernel(hidden_state, attention_mask, Wq, bq, Wk, bk, Wv, bv):
    # attention_mask: per-(batch, query) additive constant -> cancels in
    # softmax (see module docstring); unused.
    nc = build()
    in_maps = make_in_maps(hidden_state, Wq, bq, Wk, bk, Wv, bv)
    res = run_bass_kernel_spmd(nc, in_maps, list(range(N_CORES)))
    return np.stack([res.results[i]["out"] for i in range(N_CORES)], axis=0)
